# revision 28
# baseline (speedup 1.0000x reference)
"""Self-contained Trainium2 Bass kernel for nn_CalculateFlow.

Block-matching optical flow: binomial blur + u8 quantize, 7x7 SAD search with
5x5 templates (spiral tie-break argmin), Lucas-Kanade subpixel refinement on
the template ring, 3x3 median filter. Sharded row-wise across 8 NeuronCores
(68 rows/core + halos), fully data-parallel.

kernel(f_img, g_img) takes the full [1,1,544,960] fp32 inputs and returns the
full [1,2,544,960] fp32 flow.

Engine budget per sweep iteration (49 iterations):
  PE   : 6 SAD matmuls (fp16, H5 folded via shifted accumulation) + 20 ring
         matmuls (fp16 products, fp32 PSUM accumulation)
  DVE  : diff, px, py (fp16 2x mode), running min, 2 predicated copies
  Act  : |diff|, t = cost + bias (PSUM read)
  Pool : st2 partial sum, winner mask (is_equal)
"""
import numpy as np
import concourse.bass as bass
import concourse.bacc as bacc
import concourse.mybir as mybir
from concourse.tile import TileContext
from concourse import bass_utils

H, W = 544, 960
RPC = 68
W2 = 976
X0 = 8
AluOp = mybir.AluOpType
ActFn = mybir.ActivationFunctionType
f32d = mybir.dt.float32
f16d = mybir.dt.float16
i16d = mybir.dt.int16

LO = X0 - 2          # AD-domain col 0 (= image x -2); even
AW = W + 4           # 964 AD-domain width
MAGIC = 8388608.0


def spiral_rank():
    sr = 3
    s = 2 * sr + 1
    rank = np.zeros((s, s), np.int32)
    order = [(0, 0)]
    y = x = 0
    step, d = 1, 0
    dirs = [(0, 1), (1, 0), (0, -1), (-1, 0)]
    while len(order) < s * s:
        for _ in range(2):
            dy, dx = dirs[d % 4]
            for _ in range(step):
                y += dy; x += dx
                if abs(y) <= sr and abs(x) <= sr:
                    order.append((y, x))
            d += 1
        step += 1
    for r, (yy, xx) in enumerate(order):
        rank[yy + sr, xx + sr] = r
    return rank


def host_inputs(c, f_img, g_img):
    """Build the per-core input dict (numpy), f_img/g_img: [H, W] fp32."""
    r0 = c * RPC

    def hslice(img, top_halo, nrows):
        rows = [img[min(max(r0 - top_halo + p, 0), H - 1)] for p in range(nrows)]
        out = np.zeros((nrows, W2), np.float32)
        out[:, X0:X0 + W] = np.stack(rows)
        return out

    def rowmask(top_halo, nrows):
        r = np.arange(nrows) + r0 - top_halo
        return ((r >= 0) & (r < H)).astype(np.float32)[:, None]

    gradband = np.zeros((80, 74), np.float32)
    for p in range(74):
        r = r0 - 3 + p
        if r < 0 or r >= H:
            continue
        rm = min(max(r - 1, 0), H - 1)
        rp = min(max(r + 1, 0), H - 1)
        gradband[rp - (r0 - 6), p] += np.float32(1.0 / 32.0)
        gradband[rm - (r0 - 6), p] -= np.float32(1.0 / 32.0)

    band5 = np.zeros((74, 70), np.float32)
    band2 = np.zeros((74, 70), np.float32)
    for m in range(70):
        band5[m:m + 5, m] = 1.0
        band2[m, m] = 1.0
        band2[m + 4, m] = 1.0

    medup = np.zeros((70, 70), np.float32)
    meddn = np.zeros((70, 70), np.float32)
    for p in range(70):
        r = r0 - 1 + p
        rm = min(max(r - 1, 0), H - 1) - (r0 - 1)
        rp = min(max(r + 1, 0), H - 1) - (r0 - 1)
        rm = min(max(rm, 0), 69)
        rp = min(max(rp, 0), 69)
        medup[rm, p] = 1.0
        meddn[rp, p] = 1.0

    return {
        "f_t": hslice(f_img, 7, 82),
        "g_t": hslice(g_img, 4, 76),
        "fmask": rowmask(6, 80),
        "m74": rowmask(3, 74),
        "gradband": gradband,
        "band5h": band5.astype(np.float16),
        "band2h": band2.astype(np.float16),
        "medup": medup,
        "meddn": meddn,
    }


INPUT_SPECS = [
    ("f_t", [82, W2], f32d),
    ("g_t", [76, W2], f32d),
    ("fmask", [80, 1], f32d),
    ("m74", [74, 1], f32d),
    ("gradband", [80, 74], f32d),
    ("band5h", [74, 70], f16d),
    ("band2h", [74, 70], f16d),
    ("medup", [70, 70], f32d),
    ("meddn", [70, 70], f32d),
]


def build_kernel(tc, out_ap, in_aps, dbg=None):
    """Emit the full per-core program. in_aps: dict name->AP (DRAM)."""
    nc = tc.nc
    rank = spiral_rank()
    RNG = range

    import contextlib
    stack = contextlib.ExitStack()
    pool = stack.enter_context(tc.tile_pool(name="main", bufs=1))
    tpool = stack.enter_context(tc.tile_pool(name="tmp", bufs=1))
    ppool = stack.enter_context(tc.tile_pool(name="ps", bufs=2, space="PSUM"))
    rpool = stack.enter_context(tc.tile_pool(name="psr", bufs=1, space="PSUM"))

    def dload(name, shape, dtype):
        t = pool.tile(shape, dtype, tag=name, name=name)
        nc.sync.dma_start(t[:, :], in_aps[name])
        return t

    f_t = dload("f_t", [82, W2], f32d)
    g_t = dload("g_t", [76, W2], f32d)
    fmask = dload("fmask", [80, 1], f32d)
    m74 = dload("m74", [74, 1], f32d)
    gradband = dload("gradband", [80, 74], f32d)
    band5h = dload("band5h", [74, 70], f16d)
    band2h = dload("band2h", [74, 70], f16d)
    medup = dload("medup", [70, 70], f32d)
    meddn = dload("meddn", [70, 70], f32d)

    cA, cB = X0 - 2, X0 + W + 2   # h-blur compute region

    def blur_quant(src, nrows, mask, tagp, b2tag):
        """binomial blur (x16) + u8 quantize -> (blur2 fp32 tile, q fp16 tile)"""
        sh1 = pool.tile([nrows, W2], f32d, tag="wA")
        sh2 = pool.tile([nrows, W2], f32d, tag="wB")
        nc.sync.dma_start(sh1[:, :], src[1:nrows + 1, :])
        nc.sync.dma_start(sh2[:, :], src[2:nrows + 2, :])
        v = pool.tile([nrows, W2], f32d, tag="wC")
        nc.vector.scalar_tensor_tensor(out=v[:, :], in0=sh1[:, :], scalar=2.0,
                                       in1=src[0:nrows, :], op0=AluOp.mult,
                                       op1=AluOp.add)
        nc.vector.tensor_add(v[:, :], v[:, :], sh2[:, :])
        nc.vector.tensor_copy(v[:, X0 - 1:X0], v[:, X0:X0 + 1])
        nc.vector.tensor_copy(v[:, X0 + W:X0 + W + 1], v[:, X0 + W - 1:X0 + W])
        b2 = pool.tile([nrows, W2], f32d, tag=b2tag, name=f"b2{tagp}")
        nc.vector.memset(b2[:, :], 0.0)
        nc.vector.scalar_tensor_tensor(out=b2[:, cA:cB], in0=v[:, cA:cB], scalar=2.0,
                                       in1=v[:, cA - 1:cB - 1], op0=AluOp.mult,
                                       op1=AluOp.add)
        nc.vector.tensor_add(b2[:, cA:cB], b2[:, cA:cB], v[:, cA + 1:cB + 1])
        nc.vector.tensor_copy(b2[:, X0 - 1:X0], b2[:, X0:X0 + 1])
        nc.vector.tensor_copy(b2[:, X0 + W:X0 + W + 1], b2[:, X0 + W - 1:X0 + W])
        # quantize: fxs = b2*(255/16) + M rounds to integer levels (RNE);
        # q = (fxs - M)*mask  ==  fxs*mask + (-M*mask)
        biasm = pool.tile([nrows, 1], f32d, tag=f"bm{tagp}", name=f"bm{tagp}")
        nc.vector.tensor_scalar_mul(biasm[:, :], mask[:, 0:1], -MAGIC)
        fxs = pool.tile([nrows, W2], f32d, tag="wD")
        nc.scalar.activation(fxs[:, :], b2[:, :], ActFn.Copy,
                             bias=MAGIC, scale=float(255.0 / 16.0))
        q = pool.tile([nrows, W2], f16d, tag=f"q{tagp}", name=f"q{tagp}")
        nc.scalar.activation(q[:, :], fxs[:, :], ActFn.Identity,
                             bias=biasm[:, 0:1], scale=mask[:, 0:1])
        nc.vector.memset(q[:, 0:X0], 0.0)
        nc.vector.memset(q[:, X0 + W:W2], 0.0)
        return b2, q

    blur2f, f16 = blur_quant(f_t, 80, fmask, "f", "b2f")
    _, g16 = blur_quant(g_t, 74, m74, "g", "wE")  # g's blur2 is transient

    # ---------------- gradients (fp16 outputs) ----------------
    # dfy via PE: gradband^T @ blur2f  -> [74, W2] (2 banks of 488)
    psg = rpool.tile([74, 2, 512], f32d, tag="rc0")
    nc.tensor.matmul(psg[:, 0, 0:488], gradband[:, :], blur2f[:, 0:488], start=True, stop=True)
    nc.tensor.matmul(psg[:, 1, 0:488], gradband[:, :], blur2f[:, 488:976], start=True, stop=True)
    dfy = pool.tile([74, W2], f16d, tag="dfy")
    nc.scalar.activation(dfy[:, :], psg[:, :, 0:488], ActFn.Copy, scale=m74[:, 0:1])
    nc.vector.memset(dfy[:, 0:X0], 0.0)
    nc.vector.memset(dfy[:, X0 + W:W2], 0.0)
    # dfx via shifts on blur2f rows 3..77 * (1/32), masked
    b2c = pool.tile([74, W2], f32d, tag="wE")
    nc.sync.dma_start(b2c[:, :], blur2f[3:77, :])
    dfx = pool.tile([74, W2], f16d, tag="dfx")
    nc.vector.memset(dfx[:, :], 0.0)
    dsub = pool.tile([74, W], f32d, tag="wF")
    nc.vector.tensor_sub(dsub[:, :], b2c[:, X0 + 1:X0 + W + 1], b2c[:, X0 - 1:X0 + W - 1])
    nc.vector.tensor_scalar(out=dfx[:, X0:X0 + W], in0=dsub[:, :], scalar1=m74[:, 0:1],
                            scalar2=float(1.0 / 32.0), op0=AluOp.mult, op1=AluOp.mult)

    if dbg is not None:
        for key, tile in [("f16", f16), ("g16", g16), ("dfx", dfx), ("dfy", dfy)]:
            if key in dbg:
                nc.gpsimd.dma_start(dbg[key], tile[:, :])

    # ---------------- f_dj shifted copies ----------------
    fdj_e = []
    for dj in RNG(-3, 4):
        te = pool.tile([74, W2], f16d, tag=f"fdj_e{dj}", name=f"fdj_e{dj}")
        nc.sync.dma_start(te[:, :], f16[3 + dj:77 + dj, :])
        fdj_e.append(te)

    # ---------------- sweep ----------------
    # double-buffered running min: min_c writes m_cur from m_prev, so the
    # winner mask (reading m_cur) never blocks the next min (WAR 2 iters away)
    mA = pool.tile([70, W], f32d, tag="mA")
    mB = pool.tile([70, W], f32d, tag="mB")
    nc.vector.memset(mA[:, :], 3.0e7)
    T2x = pool.tile([70, W], f32d, tag="T2x")
    T2y = pool.tile([70, W], f32d, tag="T2y")
    cidx = 0

    for dj in RNG(-3, 4):
        for di in RNG(-3, 4):
            r_s = int(rank[dj + 3, di + 3])
            bias = float(np.float32(r_s / 64.0 + (dj + 3) / 512.0 + (di + 3) / 4096.0))
            fs = fdj_e[dj + 3]
            base = LO + di
            # SAD: diff & abs (fp16)
            diff = tpool.tile([74, AW], f16d, tag="diff")
            nc.vector.tensor_sub(diff[:, :], fs[:, base:base + AW], g16[:, LO:LO + AW])
            ad = tpool.tile([74, AW], f16d, tag="ad", bufs=2)
            nc.scalar.activation(ad[:, :], diff[:, :], ActFn.Abs)
            st2 = tpool.tile([74, AW - 2], f16d, tag="st2", bufs=2)
            nc.gpsimd.tensor_add(st2[:, 0:AW - 2], ad[:, 0:AW - 2], ad[:, 1:AW - 1])
            # cost vbox+hbox via 3 accumulating matmuls per half:
            # H5(ad)[x] = st2[x] + st2[x+2] + ad[x+4]
            cps = ppool.tile([70, 2, 512], f32d, tag="cps")
            for half, o in enumerate((0, 480)):
                nc.tensor.matmul(cps[:, half, 0:480], band5h[:, :], st2[:, o:o + 480],
                                 start=True, stop=False)
                nc.tensor.matmul(cps[:, half, 0:480], band5h[:, :], st2[:, o + 2:o + 482],
                                 start=False, stop=False)
                nc.tensor.matmul(cps[:, half, 0:480], band5h[:, :], ad[:, o + 4:o + 484],
                                 start=False, stop=True)
            # t = cost + bias (Act, PSUM read); m = min(t, m); mask = (t == m)
            m_prev, m_cur = (mA, mB) if cidx % 2 == 0 else (mB, mA)
            cidx += 1
            t = tpool.tile([70, W], f32d, tag="tt", bufs=2)
            nc.scalar.activation(t[:, :].rearrange("p (b f) -> p b f", b=2),
                                 cps[:, :, 0:480], ActFn.Copy, bias=bias)
            nc.vector.tensor_tensor(m_cur[:, :], t[:, :], m_prev[:, :], AluOp.min)
            # winner mask without comparisons (Pool has no compare, DVE is the
            # bottleneck): d = t - m >= 0 is exactly 0 iff this candidate won;
            # distinct (cost+bias) values differ by >= 2^-12, so Exp(-1e6*d)
            # is exactly 1.0 at winners and underflows to exactly 0.0 elsewhere.
            mask = tpool.tile([70, W], f16d, tag="mask", bufs=2)
            nc.gpsimd.tensor_sub(mask[:, :], t[:, :], m_cur[:, :])
            nc.scalar.activation(mask[:, :], mask[:, :], ActFn.Exp, scale=-1.0e6)
            # LK products (fp16 2x)
            px = tpool.tile([74, AW], f16d, tag="px")
            nc.vector.tensor_mul(px[:, :], fs[:, base:base + AW], dfx[:, LO:LO + AW])
            py = tpool.tile([74, AW], f16d, tag="py")
            nc.vector.tensor_mul(py[:, :], fs[:, base:base + AW], dfy[:, LO:LO + AW])
            # ring sums via PE column-decomposed accumulation
            rcx = rpool.tile([70, 2, 512], f32d, tag="rc0")
            rcy = rpool.tile([70, 2, 512], f32d, tag="rc1")
            for half, (o0, o1) in enumerate(((0, 480), (480, 960))):
                for k in RNG(5):
                    bnd = band5h if k in (0, 4) else band2h
                    nc.tensor.matmul(rcx[:, half, 0:480], bnd[:, :], px[:, o0 + k:o1 + k],
                                     start=(k == 0), stop=(k == 4))
                for k in RNG(5):
                    bnd = band5h if k in (0, 4) else band2h
                    nc.tensor.matmul(rcy[:, half, 0:480], bnd[:, :], py[:, o0 + k:o1 + k],
                                     start=(k == 0), stop=(k == 4))
            mi = mask[:, :].bitcast(i16d)
            for hb, (o0, o1) in enumerate(((0, 480), (480, 960))):
                nc.vector.copy_predicated(T2x[:, o0:o1], mi[:, o0:o1], rcx[:, hb, 0:480])
                nc.vector.copy_predicated(T2y[:, o0:o1], mi[:, o0:o1], rcy[:, hb, 0:480])

    m = mB if cidx % 2 == 1 else mA  # final min buffer (cidx=49 -> mB)
    if dbg is not None and "m" in dbg:
        nc.sync.dma_start(dbg["m"], m[:, :])
    if dbg is not None and "T2x" in dbg:
        nc.sync.dma_start(dbg["T2x"], T2x[:, :])

    # ---------------- decode vec (exact; split Act/DVE) ----------------
    # n = m*4096 is an exact fp32 integer; di+3 = n mod 8; dj+3 = floor(n/8) mod 8.
    nq = pool.tile([70, W], f32d, tag="wA")
    nc.vector.tensor_scalar_mul(nq[:, :], m[:, :], 4096.0)
    q8 = pool.tile([70, W], f32d, tag="wB")
    nc.scalar.activation(q8[:, :], nq[:, :], ActFn.Copy, scale=0.125, bias=-0.4375)
    nc.scalar.activation(q8[:, :], q8[:, :], ActFn.Copy, bias=MAGIC)
    nc.scalar.activation(q8[:, :], q8[:, :], ActFn.Copy, bias=-MAGIC)
    di3 = pool.tile([70, W], f32d, tag="wC")
    nc.vector.scalar_tensor_tensor(out=di3[:, :], in0=q8[:, :], scalar=-8.0,
                                   in1=nq[:, :], op0=AluOp.mult, op1=AluOp.add)
    vecx = pool.tile([70, W], f32d, tag="vecx")
    nc.scalar.activation(vecx[:, :], di3[:, :], ActFn.Copy, scale=-1.0, bias=3.0)
    q64 = pool.tile([70, W], f32d, tag="wD")
    nc.scalar.activation(q64[:, :], q8[:, :], ActFn.Copy, scale=0.125, bias=-0.4375)
    nc.scalar.activation(q64[:, :], q64[:, :], ActFn.Copy, bias=MAGIC)
    nc.scalar.activation(q64[:, :], q64[:, :], ActFn.Copy, bias=-MAGIC)
    dj3 = pool.tile([70, W], f32d, tag="wE2")
    nc.vector.scalar_tensor_tensor(out=dj3[:, :], in0=q64[:, :], scalar=-8.0,
                                   in1=q8[:, :], op0=AluOp.mult, op1=AluOp.add)
    vecy = pool.tile([70, W], f32d, tag="vecy")
    nc.scalar.activation(vecy[:, :], dj3[:, :], ActFn.Copy, scale=-1.0, bias=3.0)

    # ---------------- LK fixed part ----------------
    def ringsum_pe(prod, name):
        ps = rpool.tile([70, 2, 512], f32d, tag="rc0", name=f"ps_{name}")
        for half, (o0, o1) in enumerate(((0, 480), (480, 960))):
            for k in RNG(5):
                bnd = band5h if k in (0, 4) else band2h
                nc.tensor.matmul(ps[:, half, 0:480], bnd[:, :], prod[:, o0 + k:o1 + k],
                                 start=(k == 0), stop=(k == 4))
        sb = pool.tile([70, W], f32d, tag=f"rs_{name}", name=f"rs_{name}")
        nc.scalar.activation(sb[:, :].rearrange("p (b f) -> p b f", b=2),
                             ps[:, :, 0:480], ActFn.Copy)
        return sb

    prod = tpool.tile([74, AW], f16d, tag="px")
    nc.scalar.activation(prod[:, :], dfx[:, LO:LO + AW], ActFn.Square)
    a_rs = ringsum_pe(prod, "a")
    prod2 = tpool.tile([74, AW], f16d, tag="py")
    nc.vector.tensor_mul(prod2[:, :], dfx[:, LO:LO + AW], dfy[:, LO:LO + AW])
    b_rs = ringsum_pe(prod2, "b")
    prod3 = tpool.tile([74, AW], f16d, tag="px")
    nc.scalar.activation(prod3[:, :], dfy[:, LO:LO + AW], ActFn.Square)
    d_rs = ringsum_pe(prod3, "d")
    prod4 = tpool.tile([74, AW], f16d, tag="py")
    nc.vector.tensor_mul(prod4[:, :], g16[:, LO:LO + AW], dfx[:, LO:LO + AW])
    t1x = ringsum_pe(prod4, "t1x")
    prod5 = tpool.tile([74, AW], f16d, tag="px")
    nc.vector.tensor_mul(prod5[:, :], g16[:, LO:LO + AW], dfy[:, LO:LO + AW])
    t1y = ringsum_pe(prod5, "t1y")

    # p = (t1x - T2x)/255, q = (t1y - T2y)/255; the 1/255 is folded into rdet.
    p_ = pool.tile([70, W], f32d, tag="p_")
    nc.vector.tensor_sub(p_[:, :], t1x[:, :], T2x[:, :])
    q_ = pool.tile([70, W], f32d, tag="q_")
    nc.vector.tensor_sub(q_[:, :], t1y[:, :], T2y[:, :])

    det = pool.tile([70, W], f32d, tag="det")
    nc.vector.tensor_mul(det[:, :], a_rs[:, :], d_rs[:, :])
    bsq = tpool.tile([70, W], f32d, tag="tA")
    nc.scalar.activation(bsq[:, :], b_rs[:, :], ActFn.Square)
    nc.vector.tensor_sub(det[:, :], det[:, :], bsq[:, :])
    safe = tpool.tile([70, W], f32d, tag="tB")
    nc.vector.tensor_scalar(out=safe[:, :], in0=det[:, :], scalar1=1e-7,
                            scalar2=255.0, op0=AluOp.max, op1=AluOp.mult)
    rdet = tpool.tile([70, W], f32d, tag="rdet")
    nc.vector.reciprocal(rdet[:, :], safe[:, :])
    valid = tpool.tile([70, W], f32d, tag="valid")
    nc.vector.tensor_scalar(out=valid[:, :], in0=det[:, :], scalar1=1e-7, scalar2=None,
                            op0=AluOp.is_gt)

    def subcomp(c1, t1, c2, t2, name, eng):
        # (c1*t1 - c2*t2) * rdet, gated by valid & |u| < 1
        u = tpool.tile([70, W], f32d, tag="tU", name=f"u_{name}")
        eng.tensor_mul(u[:, :], c1[:, :], t1[:, :])
        v = tpool.tile([70, W], f32d, tag="tA", name=f"v_{name}")
        eng.tensor_mul(v[:, :], c2[:, :], t2[:, :])
        eng.tensor_sub(u[:, :], u[:, :], v[:, :])
        eng.tensor_mul(u[:, :], u[:, :], rdet[:, :])
        usq = tpool.tile([70, W], f32d, tag="usq", name=f"usq_{name}")
        nc.scalar.activation(usq[:, :], u[:, :], ActFn.Square)
        au = tpool.tile([70, W], f32d, tag="tB", name=f"au_{name}")
        nc.vector.scalar_tensor_tensor(out=au[:, :], in0=usq[:, :], scalar=1.0,
                                       in1=valid[:, :], op0=AluOp.is_lt, op1=AluOp.mult)
        eng.tensor_mul(u[:, :], u[:, :], au[:, :])
        return u

    sub_u = subcomp(d_rs, p_, b_rs, q_, "su", nc.vector)
    sub_v = subcomp(a_rs, q_, b_rs, p_, "sv", nc.vector)

    flow_u = pool.tile([70, W + 2], f32d, tag="flow_u")
    flow_v = pool.tile([70, W + 2], f32d, tag="flow_v")
    nc.vector.tensor_add(flow_u[:, 1:W + 1], vecx[:, :], sub_u[:, :])
    nc.vector.tensor_add(flow_v[:, 1:W + 1], vecy[:, :], sub_v[:, :])
    for fl, eng in ((flow_u, nc.vector), (flow_v, nc.vector)):
        eng.tensor_copy(fl[:, 0:1], fl[:, 1:2])
        eng.tensor_copy(fl[:, W + 1:W + 2], fl[:, W:W + 1])

    if dbg is not None and "flow_v" in dbg:
        nc.sync.dma_start(dbg["flow_v"], flow_v[:, :])

    # ---------------- median ----------------
    def median(fl, name, out_slice, eng):
        # row shifts via PE bands (fp32, exact single-coeff rows)
        WP = W + 2
        pu = rpool.tile([70, 2, 512], f32d, tag="rc0", name=f"pu_{name}")
        nc.tensor.matmul(pu[:, 0, 0:481], medup[:, :], fl[:, 0:481], start=True, stop=True)
        nc.tensor.matmul(pu[:, 1, 0:481], medup[:, :], fl[:, 481:WP], start=True, stop=True)
        up = pool.tile([70, WP], f32d, tag="wA", name=f"up_{name}")
        nc.scalar.copy(up[:, :], pu[:, :, 0:481])
        pd = rpool.tile([70, 2, 512], f32d, tag="rc1", name=f"pd_{name}")
        nc.tensor.matmul(pd[:, 0, 0:481], meddn[:, :], fl[:, 0:481], start=True, stop=True)
        nc.tensor.matmul(pd[:, 1, 0:481], meddn[:, :], fl[:, 481:WP], start=True, stop=True)
        dn = pool.tile([70, WP], f32d, tag="wB", name=f"dn_{name}")
        nc.scalar.copy(dn[:, :], pd[:, :, 0:481])
        A, B, C = up, fl, dn
        lo3 = tpool.tile([70, WP], f32d, tag="lo3")
        hi3 = tpool.tile([70, WP], f32d, tag="hi3")
        md3 = tpool.tile([70, WP], f32d, tag="md3")
        tmn = tpool.tile([70, WP], f32d, tag="tmn")
        eng.tensor_tensor(tmn[:, :], A[:, :], B[:, :], AluOp.min)
        eng.tensor_tensor(hi3[:, :], A[:, :], B[:, :], AluOp.max)
        eng.tensor_tensor(lo3[:, :], tmn[:, :], C[:, :], AluOp.min)
        eng.tensor_tensor(md3[:, :], hi3[:, :], C[:, :], AluOp.min)
        eng.tensor_tensor(md3[:, :], md3[:, :], tmn[:, :], AluOp.max)
        eng.tensor_tensor(hi3[:, :], hi3[:, :], C[:, :], AluOp.max)
        mx = tpool.tile([70, W], f32d, tag="tU")
        eng.tensor_tensor(mx[:, :], hi3[:, 0:W], hi3[:, 1:W + 1], AluOp.min)
        eng.tensor_tensor(mx[:, :], mx[:, :], hi3[:, 2:W + 2], AluOp.min)
        mn = tpool.tile([70, W], f32d, tag="tA")
        eng.tensor_tensor(mn[:, :], lo3[:, 0:W], lo3[:, 1:W + 1], AluOp.max)
        eng.tensor_tensor(mn[:, :], mn[:, :], lo3[:, 2:W + 2], AluOp.max)
        m2n = tpool.tile([70, W], f32d, tag="tB")
        m2x = tpool.tile([70, W], f32d, tag="rdet2")
        eng.tensor_tensor(m2n[:, :], md3[:, 0:W], md3[:, 1:W + 1], AluOp.min)
        eng.tensor_tensor(m2x[:, :], md3[:, 0:W], md3[:, 1:W + 1], AluOp.max)
        mdm = tpool.tile([70, W], f32d, tag="valid2")
        eng.tensor_tensor(mdm[:, :], m2x[:, :], md3[:, 2:W + 2], AluOp.min)
        eng.tensor_tensor(mdm[:, :], mdm[:, :], m2n[:, :], AluOp.max)
        f1 = tpool.tile([70, W], f32d, tag="f1")
        f2 = tpool.tile([70, W], f32d, tag="f2")
        eng.tensor_tensor(f1[:, :], mx[:, :], mdm[:, :], AluOp.min)
        eng.tensor_tensor(f2[:, :], mx[:, :], mdm[:, :], AluOp.max)
        eng.tensor_tensor(f2[:, :], f2[:, :], mn[:, :], AluOp.min)
        eng.tensor_tensor(f2[:, :], f2[:, :], f1[:, :], AluOp.max)
        nc.sync.dma_start(out_slice, f2[1:69, :])

    median(flow_v, "v", out_ap[0, :, :], nc.vector)
    median(flow_u, "u", out_ap[1, :, :], nc.vector)

    stack.close()


# ---------------------------------------------------------------------------
_CACHE = {}


def _get_runner(n_cores=8):
    """Build the Bass module once and return a cached jitted SPMD callable."""
    if "runner" in _CACHE:
        return _CACHE["runner"]
    import jax
    from jax.sharding import Mesh, PartitionSpec
    from jax.experimental.shard_map import shard_map
    from concourse import bass2jax

    nc = bacc.Bacc("TRN2", num_devices=n_cores)
    in_aps = {}
    for name, shape, dtype in INPUT_SPECS:
        in_aps[name] = nc.dram_tensor(name, shape, dtype, kind="ExternalInput").ap()
    out_t = nc.dram_tensor("flow_out", [2, RPC, W], mybir.dt.float32,
                           kind="ExternalOutput")
    with TileContext(nc) as tc:
        build_kernel(tc, out_t.ap(), in_aps)
    nc.compile()

    bass2jax.install_neuronx_cc_hook()
    partition_name = nc.partition_id_tensor.name if nc.partition_id_tensor else None
    in_names, out_names, out_avals, zero_shapes = [], [], [], []
    for alloc in nc.m.functions[0].allocations:
        if not isinstance(alloc, mybir.MemoryLocationSet):
            continue
        name = alloc.memorylocations[0].name
        if alloc.kind == "ExternalInput":
            if name != partition_name:
                in_names.append(name)
        elif alloc.kind == "ExternalOutput":
            out_names.append(name)
            shape = tuple(alloc.tensor_shape)
            dtype = mybir.dt.np(alloc.dtype)
            out_avals.append(jax.core.ShapedArray(shape, dtype))
            zero_shapes.append((shape, dtype))
    n_params = len(in_names)
    all_names = list(in_names) + list(out_names)
    if partition_name is not None:
        all_names.append(partition_name)
    donate = tuple(range(n_params, n_params + len(out_names)))

    def _body(*args):
        operands = list(args)
        if partition_name is not None:
            operands.append(bass2jax.partition_id_tensor())
        outs = bass2jax._bass_exec_p.bind(
            *operands,
            out_avals=tuple(out_avals),
            in_names=tuple(all_names),
            out_names=tuple(out_names),
            lowering_input_output_aliases=(),
            sim_require_finite=True,
            sim_require_nnan=True,
            nc=nc,
        )
        return tuple(outs)

    devices = jax.devices()[:n_cores]
    mesh = Mesh(np.asarray(devices), ("core",))
    in_specs = (PartitionSpec("core"),) * (n_params + len(out_names))
    out_specs = (PartitionSpec("core"),) * len(out_names)
    sharded = jax.jit(
        shard_map(_body, mesh=mesh, in_specs=in_specs, out_specs=out_specs,
                  check_rep=False),
        donate_argnums=donate, keep_unused=True,
    )
    runner = {
        "fn": sharded, "in_names": in_names, "out_names": out_names,
        "zero_shapes": zero_shapes, "n_cores": n_cores,
    }
    _CACHE["runner"] = runner
    return runner


def _concat_inputs(runner, in_maps):
    n_cores = runner["n_cores"]
    return [
        np.concatenate([np.asarray(in_maps[c][nm]) for c in range(n_cores)], axis=0)
        for nm in runner["in_names"]
    ]


def _zero_outs(runner):
    n_cores = runner["n_cores"]
    return [np.zeros((n_cores * s[0], *s[1:]), d) for s, d in runner["zero_shapes"]]


def kernel(f_img, g_img):
    f_img = np.ascontiguousarray(np.asarray(f_img), dtype=np.float32)
    g_img = np.ascontiguousarray(np.asarray(g_img), dtype=np.float32)
    assert f_img.shape == (1, 1, H, W) and g_img.shape == (1, 1, H, W)
    runner = _get_runner(8)
    f2, g2 = f_img[0, 0], g_img[0, 0]
    in_maps = [host_inputs(c, f2, g2) for c in range(8)]
    concat_in = _concat_inputs(runner, in_maps)
    outs = runner["fn"](*concat_in, *_zero_outs(runner))
    flow = np.asarray(outs[0]).reshape(8, 2, RPC, W)
    out = np.concatenate([flow[c] for c in range(8)], axis=1)
    return out[None].astype(np.float32)


# revision 29
# speedup vs baseline: 1.0390x; 1.0390x over previous
"""Self-contained Trainium2 Bass kernel for nn_CalculateFlow.

Block-matching optical flow: binomial blur + u8 quantize, 7x7 SAD search with
5x5 templates (spiral tie-break argmin), Lucas-Kanade subpixel refinement on
the template ring, 3x3 median filter. Sharded row-wise across 8 NeuronCores
(68 rows/core + halos), fully data-parallel.

kernel(f_img, g_img) takes the full [1,1,544,960] fp32 inputs and returns the
full [1,2,544,960] fp32 flow.

Engine budget per sweep iteration (49 iterations):
  PE   : 6 SAD matmuls (fp16, H5 folded via shifted accumulation) + 20 ring
         matmuls (fp16 products, fp32 PSUM accumulation)
  DVE  : diff, px, py (fp16 2x mode), running min, 2 predicated copies
  Act  : |diff|, t = cost + bias (PSUM read)
  Pool : st2 partial sum, winner mask (is_equal)
"""
import numpy as np
import concourse.bass as bass
import concourse.bacc as bacc
import concourse.mybir as mybir
from concourse.tile import TileContext
from concourse import bass_utils

H, W = 544, 960
RPC = 68
W2 = 976
X0 = 8
AluOp = mybir.AluOpType
ActFn = mybir.ActivationFunctionType
f32d = mybir.dt.float32
f16d = mybir.dt.float16
i16d = mybir.dt.int16

LO = X0 - 2          # AD-domain col 0 (= image x -2); even
AW = W + 4           # 964 AD-domain width
MAGIC = 8388608.0


def spiral_rank():
    sr = 3
    s = 2 * sr + 1
    rank = np.zeros((s, s), np.int32)
    order = [(0, 0)]
    y = x = 0
    step, d = 1, 0
    dirs = [(0, 1), (1, 0), (0, -1), (-1, 0)]
    while len(order) < s * s:
        for _ in range(2):
            dy, dx = dirs[d % 4]
            for _ in range(step):
                y += dy; x += dx
                if abs(y) <= sr and abs(x) <= sr:
                    order.append((y, x))
            d += 1
        step += 1
    for r, (yy, xx) in enumerate(order):
        rank[yy + sr, xx + sr] = r
    return rank


def host_inputs(c, f_img, g_img):
    """Build the per-core input dict (numpy), f_img/g_img: [H, W] fp32."""
    r0 = c * RPC

    def hslice(img, top_halo, nrows):
        rows = [img[min(max(r0 - top_halo + p, 0), H - 1)] for p in range(nrows)]
        out = np.zeros((nrows, W2), np.float32)
        out[:, X0:X0 + W] = np.stack(rows)
        return out

    def rowmask(top_halo, nrows):
        r = np.arange(nrows) + r0 - top_halo
        return ((r >= 0) & (r < H)).astype(np.float32)[:, None]

    gradband = np.zeros((80, 74), np.float32)
    for p in range(74):
        r = r0 - 3 + p
        if r < 0 or r >= H:
            continue
        rm = min(max(r - 1, 0), H - 1)
        rp = min(max(r + 1, 0), H - 1)
        gradband[rp - (r0 - 6), p] += np.float32(1.0 / 32.0)
        gradband[rm - (r0 - 6), p] -= np.float32(1.0 / 32.0)

    band5 = np.zeros((74, 70), np.float32)
    band2 = np.zeros((74, 70), np.float32)
    for m in range(70):
        band5[m:m + 5, m] = 1.0
        band2[m, m] = 1.0
        band2[m + 4, m] = 1.0

    medup = np.zeros((70, 70), np.float32)
    meddn = np.zeros((70, 70), np.float32)
    for p in range(70):
        r = r0 - 1 + p
        rm = min(max(r - 1, 0), H - 1) - (r0 - 1)
        rp = min(max(r + 1, 0), H - 1) - (r0 - 1)
        rm = min(max(rm, 0), 69)
        rp = min(max(rp, 0), 69)
        medup[rm, p] = 1.0
        meddn[rp, p] = 1.0

    return {
        "f_t": hslice(f_img, 7, 82),
        "g_t": hslice(g_img, 4, 76),
        "fmask": rowmask(6, 80),
        "m74": rowmask(3, 74),
        "gradband": gradband,
        "band5h": band5.astype(np.float16),
        "band2h": band2.astype(np.float16),
        "medup": medup,
        "meddn": meddn,
    }


INPUT_SPECS = [
    ("f_t", [82, W2], f32d),
    ("g_t", [76, W2], f32d),
    ("fmask", [80, 1], f32d),
    ("m74", [74, 1], f32d),
    ("gradband", [80, 74], f32d),
    ("band5h", [74, 70], f16d),
    ("band2h", [74, 70], f16d),
    ("medup", [70, 70], f32d),
    ("meddn", [70, 70], f32d),
]


def build_kernel(tc, out_ap, in_aps, dbg=None):
    """Emit the full per-core program. in_aps: dict name->AP (DRAM)."""
    nc = tc.nc
    rank = spiral_rank()
    RNG = range

    import contextlib
    stack = contextlib.ExitStack()
    pool = stack.enter_context(tc.tile_pool(name="main", bufs=1))
    tpool = stack.enter_context(tc.tile_pool(name="tmp", bufs=1))
    ppool = stack.enter_context(tc.tile_pool(name="ps", bufs=2, space="PSUM"))
    rpool = stack.enter_context(tc.tile_pool(name="psr", bufs=1, space="PSUM"))

    def dload(name, shape, dtype):
        t = pool.tile(shape, dtype, tag=name, name=name)
        nc.sync.dma_start(t[:, :], in_aps[name])
        return t

    f_t = dload("f_t", [82, W2], f32d)
    g_t = dload("g_t", [76, W2], f32d)
    fmask = dload("fmask", [80, 1], f32d)
    m74 = dload("m74", [74, 1], f32d)
    gradband = dload("gradband", [80, 74], f32d)
    band5h = dload("band5h", [74, 70], f16d)
    band2h = dload("band2h", [74, 70], f16d)
    medup = dload("medup", [70, 70], f32d)
    meddn = dload("meddn", [70, 70], f32d)

    cA, cB = X0 - 2, X0 + W + 2   # h-blur compute region

    def blur_quant(src, nrows, mask, tagp, b2tag):
        """binomial blur (x16) + u8 quantize -> (blur2 fp32 tile, q fp16 tile)"""
        sh1 = pool.tile([nrows, W2], f32d, tag="wA")
        sh2 = pool.tile([nrows, W2], f32d, tag="wB")
        nc.sync.dma_start(sh1[:, :], src[1:nrows + 1, :])
        nc.sync.dma_start(sh2[:, :], src[2:nrows + 2, :])
        v = pool.tile([nrows, W2], f32d, tag="wC")
        nc.vector.scalar_tensor_tensor(out=v[:, :], in0=sh1[:, :], scalar=2.0,
                                       in1=src[0:nrows, :], op0=AluOp.mult,
                                       op1=AluOp.add)
        nc.vector.tensor_add(v[:, :], v[:, :], sh2[:, :])
        nc.vector.tensor_copy(v[:, X0 - 1:X0], v[:, X0:X0 + 1])
        nc.vector.tensor_copy(v[:, X0 + W:X0 + W + 1], v[:, X0 + W - 1:X0 + W])
        b2 = pool.tile([nrows, W2], f32d, tag=b2tag, name=f"b2{tagp}")
        nc.vector.memset(b2[:, :], 0.0)
        nc.vector.scalar_tensor_tensor(out=b2[:, cA:cB], in0=v[:, cA:cB], scalar=2.0,
                                       in1=v[:, cA - 1:cB - 1], op0=AluOp.mult,
                                       op1=AluOp.add)
        nc.vector.tensor_add(b2[:, cA:cB], b2[:, cA:cB], v[:, cA + 1:cB + 1])
        nc.vector.tensor_copy(b2[:, X0 - 1:X0], b2[:, X0:X0 + 1])
        nc.vector.tensor_copy(b2[:, X0 + W:X0 + W + 1], b2[:, X0 + W - 1:X0 + W])
        # quantize: fxs = b2*(255/16) + M rounds to integer levels (RNE);
        # q = (fxs - M)*mask  ==  fxs*mask + (-M*mask)
        biasm = pool.tile([nrows, 1], f32d, tag=f"bm{tagp}", name=f"bm{tagp}")
        nc.vector.tensor_scalar_mul(biasm[:, :], mask[:, 0:1], -MAGIC)
        fxs = pool.tile([nrows, W2], f32d, tag="wD")
        nc.scalar.activation(fxs[:, :], b2[:, :], ActFn.Copy,
                             bias=MAGIC, scale=float(255.0 / 16.0))
        q = pool.tile([nrows, W2], f16d, tag=f"q{tagp}", name=f"q{tagp}")
        nc.scalar.activation(q[:, :], fxs[:, :], ActFn.Identity,
                             bias=biasm[:, 0:1], scale=mask[:, 0:1])
        nc.vector.memset(q[:, 0:X0], 0.0)
        nc.vector.memset(q[:, X0 + W:W2], 0.0)
        return b2, q

    blur2f, f16 = blur_quant(f_t, 80, fmask, "f", "b2f")
    _, g16 = blur_quant(g_t, 74, m74, "g", "wE")  # g's blur2 is transient

    # ---------------- gradients (fp16 outputs) ----------------
    # dfy via PE: gradband^T @ blur2f  -> [74, W2] (2 banks of 488)
    psg = rpool.tile([74, 2, 512], f32d, tag="rc0")
    nc.tensor.matmul(psg[:, 0, 0:488], gradband[:, :], blur2f[:, 0:488], start=True, stop=True)
    nc.tensor.matmul(psg[:, 1, 0:488], gradband[:, :], blur2f[:, 488:976], start=True, stop=True)
    dfy = pool.tile([74, W2], f16d, tag="dfy")
    nc.scalar.activation(dfy[:, :], psg[:, :, 0:488], ActFn.Copy, scale=m74[:, 0:1])
    nc.vector.memset(dfy[:, 0:X0], 0.0)
    nc.vector.memset(dfy[:, X0 + W:W2], 0.0)
    # dfx via shifts on blur2f rows 3..77 * (1/32), masked
    b2c = pool.tile([74, W2], f32d, tag="wE")
    nc.sync.dma_start(b2c[:, :], blur2f[3:77, :])
    dfx = pool.tile([74, W2], f16d, tag="dfx")
    nc.vector.memset(dfx[:, :], 0.0)
    dsub = pool.tile([74, W], f32d, tag="wF")
    nc.vector.tensor_sub(dsub[:, :], b2c[:, X0 + 1:X0 + W + 1], b2c[:, X0 - 1:X0 + W - 1])
    nc.vector.tensor_scalar(out=dfx[:, X0:X0 + W], in0=dsub[:, :], scalar1=m74[:, 0:1],
                            scalar2=float(1.0 / 32.0), op0=AluOp.mult, op1=AluOp.mult)

    if dbg is not None:
        for key, tile in [("f16", f16), ("g16", g16), ("dfx", dfx), ("dfy", dfy)]:
            if key in dbg:
                nc.gpsimd.dma_start(dbg[key], tile[:, :])

    # ---------------- f_dj shifted copies ----------------
    fdj_e = []
    for dj in RNG(-3, 4):
        te = pool.tile([74, W2], f16d, tag=f"fdj_e{dj}", name=f"fdj_e{dj}")
        nc.sync.dma_start(te[:, :], f16[3 + dj:77 + dj, :])
        fdj_e.append(te)

    # ---------------- sweep ----------------
    # double-buffered running min: min_c writes m_cur from m_prev, so the
    # winner mask (reading m_cur) never blocks the next min (WAR 2 iters away)
    mA = pool.tile([70, W], f32d, tag="mA")
    mB = pool.tile([70, W], f32d, tag="mB")
    nc.vector.memset(mA[:, :], 3.0e7)
    T2x = pool.tile([70, W], f32d, tag="T2x")
    T2y = pool.tile([70, W], f32d, tag="T2y")
    cidx = 0

    for dj in RNG(-3, 4):
        for di in RNG(-3, 4):
            r_s = int(rank[dj + 3, di + 3])
            bias = float(np.float32(r_s / 64.0 + (dj + 3) / 512.0 + (di + 3) / 4096.0))
            fs = fdj_e[dj + 3]
            base = LO + di
            # SAD: diff & abs (fp16)
            diff = tpool.tile([74, AW], f16d, tag="diff")
            nc.gpsimd.tensor_sub(diff[:, :], fs[:, base:base + AW], g16[:, LO:LO + AW])
            ad = tpool.tile([74, AW], f16d, tag="ad", bufs=2)
            nc.scalar.activation(ad[:, :], diff[:, :], ActFn.Abs)
            st2 = tpool.tile([74, AW - 2], f16d, tag="st2", bufs=2)
            nc.gpsimd.tensor_add(st2[:, 0:AW - 2], ad[:, 0:AW - 2], ad[:, 1:AW - 1])
            # cost vbox+hbox via 3 accumulating matmuls per half:
            # H5(ad)[x] = st2[x] + st2[x+2] + ad[x+4]
            cps = ppool.tile([70, 2, 512], f32d, tag="cps")
            for half, o in enumerate((0, 480)):
                nc.tensor.matmul(cps[:, half, 0:480], band5h[:, :], st2[:, o:o + 480],
                                 start=True, stop=False)
                nc.tensor.matmul(cps[:, half, 0:480], band5h[:, :], st2[:, o + 2:o + 482],
                                 start=False, stop=False)
                nc.tensor.matmul(cps[:, half, 0:480], band5h[:, :], ad[:, o + 4:o + 484],
                                 start=False, stop=True)
            # t = cost + bias (Act, PSUM read); m = min(t, m); mask = (t == m)
            m_prev, m_cur = (mA, mB) if cidx % 2 == 0 else (mB, mA)
            cidx += 1
            t = tpool.tile([70, W], f32d, tag="tt", bufs=2)
            nc.scalar.activation(t[:, :].rearrange("p (b f) -> p b f", b=2),
                                 cps[:, :, 0:480], ActFn.Copy, bias=bias)
            nc.vector.tensor_tensor(m_cur[:, :], t[:, :], m_prev[:, :], AluOp.min)
            # winner mask without comparisons (Pool has no compare, DVE is the
            # bottleneck): d = t - m >= 0 is exactly 0 iff this candidate won;
            # distinct (cost+bias) values differ by >= 2^-12, so Exp(-1e6*d)
            # is exactly 1.0 at winners and underflows to exactly 0.0 elsewhere.
            mask = tpool.tile([70, W], f16d, tag="mask", bufs=2)
            nc.gpsimd.tensor_sub(mask[:, :], t[:, :], m_cur[:, :])
            nc.scalar.activation(mask[:, :], mask[:, :], ActFn.Exp, scale=-1.0e6)
            # LK products (fp16 2x)
            px = tpool.tile([74, AW], f16d, tag="px")
            nc.vector.tensor_mul(px[:, :], fs[:, base:base + AW], dfx[:, LO:LO + AW])
            py = tpool.tile([74, AW], f16d, tag="py")
            nc.vector.tensor_mul(py[:, :], fs[:, base:base + AW], dfy[:, LO:LO + AW])
            # ring sums via PE column-decomposed accumulation
            rcx = rpool.tile([70, 2, 512], f32d, tag="rc0")
            rcy = rpool.tile([70, 2, 512], f32d, tag="rc1")
            for half, (o0, o1) in enumerate(((0, 480), (480, 960))):
                for k in RNG(5):
                    bnd = band5h if k in (0, 4) else band2h
                    nc.tensor.matmul(rcx[:, half, 0:480], bnd[:, :], px[:, o0 + k:o1 + k],
                                     start=(k == 0), stop=(k == 4))
                for k in RNG(5):
                    bnd = band5h if k in (0, 4) else band2h
                    nc.tensor.matmul(rcy[:, half, 0:480], bnd[:, :], py[:, o0 + k:o1 + k],
                                     start=(k == 0), stop=(k == 4))
            mi = mask[:, :].bitcast(i16d).rearrange("p (b f) -> p b f", b=2)
            nc.vector.copy_predicated(
                T2x[:, :].rearrange("p (b f) -> p b f", b=2), mi, rcx[:, :, 0:480])
            nc.vector.copy_predicated(
                T2y[:, :].rearrange("p (b f) -> p b f", b=2), mi, rcy[:, :, 0:480])

    m = mB if cidx % 2 == 1 else mA  # final min buffer (cidx=49 -> mB)
    if dbg is not None and "m" in dbg:
        nc.sync.dma_start(dbg["m"], m[:, :])
    if dbg is not None and "T2x" in dbg:
        nc.sync.dma_start(dbg["T2x"], T2x[:, :])

    # ---------------- decode vec (exact; split Act/DVE) ----------------
    # n = m*4096 is an exact fp32 integer; di+3 = n mod 8; dj+3 = floor(n/8) mod 8.
    nq = pool.tile([70, W], f32d, tag="wA")
    nc.vector.tensor_scalar_mul(nq[:, :], m[:, :], 4096.0)
    q8 = pool.tile([70, W], f32d, tag="wB")
    nc.scalar.activation(q8[:, :], nq[:, :], ActFn.Copy, scale=0.125, bias=-0.4375)
    nc.scalar.activation(q8[:, :], q8[:, :], ActFn.Copy, bias=MAGIC)
    nc.scalar.activation(q8[:, :], q8[:, :], ActFn.Copy, bias=-MAGIC)
    di3 = pool.tile([70, W], f32d, tag="wC")
    nc.vector.scalar_tensor_tensor(out=di3[:, :], in0=q8[:, :], scalar=-8.0,
                                   in1=nq[:, :], op0=AluOp.mult, op1=AluOp.add)
    vecx = pool.tile([70, W], f32d, tag="vecx")
    nc.scalar.activation(vecx[:, :], di3[:, :], ActFn.Copy, scale=-1.0, bias=3.0)
    q64 = pool.tile([70, W], f32d, tag="wD")
    nc.scalar.activation(q64[:, :], q8[:, :], ActFn.Copy, scale=0.125, bias=-0.4375)
    nc.scalar.activation(q64[:, :], q64[:, :], ActFn.Copy, bias=MAGIC)
    nc.scalar.activation(q64[:, :], q64[:, :], ActFn.Copy, bias=-MAGIC)
    dj3 = pool.tile([70, W], f32d, tag="wE2")
    nc.vector.scalar_tensor_tensor(out=dj3[:, :], in0=q64[:, :], scalar=-8.0,
                                   in1=q8[:, :], op0=AluOp.mult, op1=AluOp.add)
    vecy = pool.tile([70, W], f32d, tag="vecy")
    nc.scalar.activation(vecy[:, :], dj3[:, :], ActFn.Copy, scale=-1.0, bias=3.0)

    # ---------------- LK fixed part ----------------
    def ringsum_pe(prod, name):
        ps = rpool.tile([70, 2, 512], f32d, tag="rc0", name=f"ps_{name}")
        for half, (o0, o1) in enumerate(((0, 480), (480, 960))):
            for k in RNG(5):
                bnd = band5h if k in (0, 4) else band2h
                nc.tensor.matmul(ps[:, half, 0:480], bnd[:, :], prod[:, o0 + k:o1 + k],
                                 start=(k == 0), stop=(k == 4))
        sb = pool.tile([70, W], f32d, tag=f"rs_{name}", name=f"rs_{name}")
        nc.scalar.activation(sb[:, :].rearrange("p (b f) -> p b f", b=2),
                             ps[:, :, 0:480], ActFn.Copy)
        return sb

    prod = tpool.tile([74, AW], f16d, tag="px")
    nc.scalar.activation(prod[:, :], dfx[:, LO:LO + AW], ActFn.Square)
    a_rs = ringsum_pe(prod, "a")
    prod2 = tpool.tile([74, AW], f16d, tag="py")
    nc.vector.tensor_mul(prod2[:, :], dfx[:, LO:LO + AW], dfy[:, LO:LO + AW])
    b_rs = ringsum_pe(prod2, "b")
    prod3 = tpool.tile([74, AW], f16d, tag="px")
    nc.scalar.activation(prod3[:, :], dfy[:, LO:LO + AW], ActFn.Square)
    d_rs = ringsum_pe(prod3, "d")
    prod4 = tpool.tile([74, AW], f16d, tag="py")
    nc.vector.tensor_mul(prod4[:, :], g16[:, LO:LO + AW], dfx[:, LO:LO + AW])
    t1x = ringsum_pe(prod4, "t1x")
    prod5 = tpool.tile([74, AW], f16d, tag="px")
    nc.vector.tensor_mul(prod5[:, :], g16[:, LO:LO + AW], dfy[:, LO:LO + AW])
    t1y = ringsum_pe(prod5, "t1y")

    # p = (t1x - T2x)/255, q = (t1y - T2y)/255; the 1/255 is folded into rdet.
    p_ = pool.tile([70, W], f32d, tag="p_")
    nc.vector.tensor_sub(p_[:, :], t1x[:, :], T2x[:, :])
    q_ = pool.tile([70, W], f32d, tag="q_")
    nc.vector.tensor_sub(q_[:, :], t1y[:, :], T2y[:, :])

    det = pool.tile([70, W], f32d, tag="det")
    nc.vector.tensor_mul(det[:, :], a_rs[:, :], d_rs[:, :])
    bsq = tpool.tile([70, W], f32d, tag="tA")
    nc.scalar.activation(bsq[:, :], b_rs[:, :], ActFn.Square)
    nc.vector.tensor_sub(det[:, :], det[:, :], bsq[:, :])
    safe = tpool.tile([70, W], f32d, tag="tB")
    nc.vector.tensor_scalar(out=safe[:, :], in0=det[:, :], scalar1=1e-7,
                            scalar2=255.0, op0=AluOp.max, op1=AluOp.mult)
    rdet = tpool.tile([70, W], f32d, tag="rdet")
    nc.vector.reciprocal(rdet[:, :], safe[:, :])
    valid = tpool.tile([70, W], f32d, tag="valid")
    nc.vector.tensor_scalar(out=valid[:, :], in0=det[:, :], scalar1=1e-7, scalar2=None,
                            op0=AluOp.is_gt)

    def subcomp(c1, t1, c2, t2, name, eng):
        # (c1*t1 - c2*t2) * rdet, gated by valid & |u| < 1
        u = tpool.tile([70, W], f32d, tag="tU", name=f"u_{name}")
        eng.tensor_mul(u[:, :], c1[:, :], t1[:, :])
        v = tpool.tile([70, W], f32d, tag="tA", name=f"v_{name}")
        eng.tensor_mul(v[:, :], c2[:, :], t2[:, :])
        eng.tensor_sub(u[:, :], u[:, :], v[:, :])
        eng.tensor_mul(u[:, :], u[:, :], rdet[:, :])
        usq = tpool.tile([70, W], f32d, tag="usq", name=f"usq_{name}")
        nc.scalar.activation(usq[:, :], u[:, :], ActFn.Square)
        au = tpool.tile([70, W], f32d, tag="tB", name=f"au_{name}")
        nc.vector.scalar_tensor_tensor(out=au[:, :], in0=usq[:, :], scalar=1.0,
                                       in1=valid[:, :], op0=AluOp.is_lt, op1=AluOp.mult)
        eng.tensor_mul(u[:, :], u[:, :], au[:, :])
        return u

    sub_u = subcomp(d_rs, p_, b_rs, q_, "su", nc.vector)
    sub_v = subcomp(a_rs, q_, b_rs, p_, "sv", nc.vector)

    flow_u = pool.tile([70, W + 2], f32d, tag="flow_u")
    flow_v = pool.tile([70, W + 2], f32d, tag="flow_v")
    nc.vector.tensor_add(flow_u[:, 1:W + 1], vecx[:, :], sub_u[:, :])
    nc.vector.tensor_add(flow_v[:, 1:W + 1], vecy[:, :], sub_v[:, :])
    for fl, eng in ((flow_u, nc.vector), (flow_v, nc.vector)):
        eng.tensor_copy(fl[:, 0:1], fl[:, 1:2])
        eng.tensor_copy(fl[:, W + 1:W + 2], fl[:, W:W + 1])

    if dbg is not None and "flow_v" in dbg:
        nc.sync.dma_start(dbg["flow_v"], flow_v[:, :])

    # ---------------- median ----------------
    def median(fl, name, out_slice, eng):
        # row shifts via PE bands (fp32, exact single-coeff rows)
        WP = W + 2
        pu = rpool.tile([70, 2, 512], f32d, tag="rc0", name=f"pu_{name}")
        nc.tensor.matmul(pu[:, 0, 0:481], medup[:, :], fl[:, 0:481], start=True, stop=True)
        nc.tensor.matmul(pu[:, 1, 0:481], medup[:, :], fl[:, 481:WP], start=True, stop=True)
        up = pool.tile([70, WP], f32d, tag="wA", name=f"up_{name}")
        nc.scalar.copy(up[:, :], pu[:, :, 0:481])
        pd = rpool.tile([70, 2, 512], f32d, tag="rc1", name=f"pd_{name}")
        nc.tensor.matmul(pd[:, 0, 0:481], meddn[:, :], fl[:, 0:481], start=True, stop=True)
        nc.tensor.matmul(pd[:, 1, 0:481], meddn[:, :], fl[:, 481:WP], start=True, stop=True)
        dn = pool.tile([70, WP], f32d, tag="wB", name=f"dn_{name}")
        nc.scalar.copy(dn[:, :], pd[:, :, 0:481])
        A, B, C = up, fl, dn
        lo3 = tpool.tile([70, WP], f32d, tag="lo3")
        hi3 = tpool.tile([70, WP], f32d, tag="hi3")
        md3 = tpool.tile([70, WP], f32d, tag="md3")
        tmn = tpool.tile([70, WP], f32d, tag="tmn")
        eng.tensor_tensor(tmn[:, :], A[:, :], B[:, :], AluOp.min)
        eng.tensor_tensor(hi3[:, :], A[:, :], B[:, :], AluOp.max)
        eng.tensor_tensor(lo3[:, :], tmn[:, :], C[:, :], AluOp.min)
        eng.tensor_tensor(md3[:, :], hi3[:, :], C[:, :], AluOp.min)
        eng.tensor_tensor(md3[:, :], md3[:, :], tmn[:, :], AluOp.max)
        eng.tensor_tensor(hi3[:, :], hi3[:, :], C[:, :], AluOp.max)
        mx = tpool.tile([70, W], f32d, tag="tU")
        eng.tensor_tensor(mx[:, :], hi3[:, 0:W], hi3[:, 1:W + 1], AluOp.min)
        eng.tensor_tensor(mx[:, :], mx[:, :], hi3[:, 2:W + 2], AluOp.min)
        mn = tpool.tile([70, W], f32d, tag="tA")
        eng.tensor_tensor(mn[:, :], lo3[:, 0:W], lo3[:, 1:W + 1], AluOp.max)
        eng.tensor_tensor(mn[:, :], mn[:, :], lo3[:, 2:W + 2], AluOp.max)
        m2n = tpool.tile([70, W], f32d, tag="tB")
        m2x = tpool.tile([70, W], f32d, tag="rdet2")
        eng.tensor_tensor(m2n[:, :], md3[:, 0:W], md3[:, 1:W + 1], AluOp.min)
        eng.tensor_tensor(m2x[:, :], md3[:, 0:W], md3[:, 1:W + 1], AluOp.max)
        mdm = tpool.tile([70, W], f32d, tag="valid2")
        eng.tensor_tensor(mdm[:, :], m2x[:, :], md3[:, 2:W + 2], AluOp.min)
        eng.tensor_tensor(mdm[:, :], mdm[:, :], m2n[:, :], AluOp.max)
        f1 = tpool.tile([70, W], f32d, tag="f1")
        f2 = tpool.tile([70, W], f32d, tag="f2")
        eng.tensor_tensor(f1[:, :], mx[:, :], mdm[:, :], AluOp.min)
        eng.tensor_tensor(f2[:, :], mx[:, :], mdm[:, :], AluOp.max)
        eng.tensor_tensor(f2[:, :], f2[:, :], mn[:, :], AluOp.min)
        eng.tensor_tensor(f2[:, :], f2[:, :], f1[:, :], AluOp.max)
        nc.sync.dma_start(out_slice, f2[1:69, :])

    median(flow_v, "v", out_ap[0, :, :], nc.vector)
    median(flow_u, "u", out_ap[1, :, :], nc.vector)

    stack.close()


# ---------------------------------------------------------------------------
_CACHE = {}


def _get_runner(n_cores=8):
    """Build the Bass module once and return a cached jitted SPMD callable."""
    if "runner" in _CACHE:
        return _CACHE["runner"]
    import jax
    from jax.sharding import Mesh, PartitionSpec
    from jax.experimental.shard_map import shard_map
    from concourse import bass2jax

    nc = bacc.Bacc("TRN2", num_devices=n_cores)
    in_aps = {}
    for name, shape, dtype in INPUT_SPECS:
        in_aps[name] = nc.dram_tensor(name, shape, dtype, kind="ExternalInput").ap()
    out_t = nc.dram_tensor("flow_out", [2, RPC, W], mybir.dt.float32,
                           kind="ExternalOutput")
    with TileContext(nc) as tc:
        build_kernel(tc, out_t.ap(), in_aps)
    nc.compile()

    bass2jax.install_neuronx_cc_hook()
    partition_name = nc.partition_id_tensor.name if nc.partition_id_tensor else None
    in_names, out_names, out_avals, zero_shapes = [], [], [], []
    for alloc in nc.m.functions[0].allocations:
        if not isinstance(alloc, mybir.MemoryLocationSet):
            continue
        name = alloc.memorylocations[0].name
        if alloc.kind == "ExternalInput":
            if name != partition_name:
                in_names.append(name)
        elif alloc.kind == "ExternalOutput":
            out_names.append(name)
            shape = tuple(alloc.tensor_shape)
            dtype = mybir.dt.np(alloc.dtype)
            out_avals.append(jax.core.ShapedArray(shape, dtype))
            zero_shapes.append((shape, dtype))
    n_params = len(in_names)
    all_names = list(in_names) + list(out_names)
    if partition_name is not None:
        all_names.append(partition_name)
    donate = tuple(range(n_params, n_params + len(out_names)))

    def _body(*args):
        operands = list(args)
        if partition_name is not None:
            operands.append(bass2jax.partition_id_tensor())
        outs = bass2jax._bass_exec_p.bind(
            *operands,
            out_avals=tuple(out_avals),
            in_names=tuple(all_names),
            out_names=tuple(out_names),
            lowering_input_output_aliases=(),
            sim_require_finite=True,
            sim_require_nnan=True,
            nc=nc,
        )
        return tuple(outs)

    devices = jax.devices()[:n_cores]
    mesh = Mesh(np.asarray(devices), ("core",))
    in_specs = (PartitionSpec("core"),) * (n_params + len(out_names))
    out_specs = (PartitionSpec("core"),) * len(out_names)
    sharded = jax.jit(
        shard_map(_body, mesh=mesh, in_specs=in_specs, out_specs=out_specs,
                  check_rep=False),
        donate_argnums=donate, keep_unused=True,
    )
    runner = {
        "fn": sharded, "in_names": in_names, "out_names": out_names,
        "zero_shapes": zero_shapes, "n_cores": n_cores,
    }
    _CACHE["runner"] = runner
    return runner


def _concat_inputs(runner, in_maps):
    n_cores = runner["n_cores"]
    return [
        np.concatenate([np.asarray(in_maps[c][nm]) for c in range(n_cores)], axis=0)
        for nm in runner["in_names"]
    ]


def _zero_outs(runner):
    n_cores = runner["n_cores"]
    return [np.zeros((n_cores * s[0], *s[1:]), d) for s, d in runner["zero_shapes"]]


def kernel(f_img, g_img):
    f_img = np.ascontiguousarray(np.asarray(f_img), dtype=np.float32)
    g_img = np.ascontiguousarray(np.asarray(g_img), dtype=np.float32)
    assert f_img.shape == (1, 1, H, W) and g_img.shape == (1, 1, H, W)
    runner = _get_runner(8)
    f2, g2 = f_img[0, 0], g_img[0, 0]
    in_maps = [host_inputs(c, f2, g2) for c in range(8)]
    concat_in = _concat_inputs(runner, in_maps)
    outs = runner["fn"](*concat_in, *_zero_outs(runner))
    flow = np.asarray(outs[0]).reshape(8, 2, RPC, W)
    out = np.concatenate([flow[c] for c in range(8)], axis=1)
    return out[None].astype(np.float32)
